# revision 46
# baseline (speedup 1.0000x reference)
"""Trainium2 Bass kernel for nn_CE_25872882991735 — v4b reconstruction.

Best-measured variant (166.5us). Phase 1: f32 loads -> ACT bf16 cast with
rowsum accum -> regular-matmul transposes (HAM-warm) -> bf16 Gram per image.
Single 68KB AllReduce; fp32 Newton-Schulz fused both halves; bf16 fused
apply; bf16 output.
"""
import sys

try:
    import concourse.bass as bass  # noqa: F401
except ImportError:  # pragma: no cover
    sys.path.insert(0, "/opt/trn_rl_repo")

import numpy as np

import concourse.bacc as bacc
import concourse.tile as tile
from concourse import mybir
from concourse import bass_utils

F32 = mybir.dt.float32
BF16 = mybir.dt.bfloat16
AX = mybir.AxisListType
ALU = mybir.AluOpType
ACTF = mybir.ActivationFunctionType

N_CORES = 8
EPS = 1e-5
LN_EPS = 1e-5
T_NEWTON = 3

_CRIT_COLS = {}
_REST_COLS = {}


def _build_cols():
    c = 0
    for name, w in [("ident", 128), ("onesrow", 128), ("xw", 1)]:
        _CRIT_COLS[name] = (c, c + w)
        c += w
    cw_crit = c
    c = 0
    for name, w in [("neghalf", 128), ("maskeps2", 256), ("ioverm2", 256),
                    ("fc1t", 128), ("fc2t", 256), ("gmask", 2),
                    ("gmaskT", 128), ("gmaskT15", 128), ("ones", 1),
                    ("lng", 64), ("lnb", 64)]:
        _REST_COLS[name] = (c, c + w)
        c += w
    return cw_crit, c


CW_CRIT, CW_REST = _build_cols()


def _consts_pack(fc1_w, fc2_w, ln_g, ln_b, x_weight, m_total):
    cpc = np.zeros((128, CW_CRIT), np.float32)
    cpr = np.zeros((128, CW_REST), np.float32)

    def putc(name, arr):
        c0, c1 = _CRIT_COLS[name]
        cpc[:arr.shape[0], c0:c1] = arr

    def putr(name, arr):
        c0, c1 = _REST_COLS[name]
        cpr[:arr.shape[0], c0:c1] = arr

    ident = np.eye(128, dtype=np.float32)
    putc("ident", ident)
    putc("onesrow", np.ones((1, 128), np.float32))
    putc("xw", np.asarray(x_weight, np.float32).reshape(1, 1))

    putr("neghalf", (-0.5 * ident).astype(np.float32))
    blk = np.zeros((128, 128), np.float32)
    blk[:64, :64] = EPS
    blk[64:, 64:] = EPS
    putr("maskeps2", np.concatenate([blk, blk], axis=1))
    iov = ident * (1.0 / m_total)
    putr("ioverm2", np.concatenate([iov, iov], axis=1))
    f1 = np.ascontiguousarray(fc1_w.T).reshape(2, 128, 64)
    f1p = np.zeros((128, 128), np.float32)
    f1p[:, 0:64] = f1[0]
    f1p[:, 64:128] = f1[1]
    putr("fc1t", f1p)
    f2 = np.zeros((64, 256), np.float32)
    f2[:, :] = fc2_w.T
    putr("fc2t", f2)
    gmask = np.zeros((128, 2), np.float32)
    gmask[:64, 0] = 1.0
    gmask[64:, 1] = 1.0
    putr("gmask", gmask)
    putr("gmaskT", gmask.T)
    putr("gmaskT15", (1.5 * gmask.T).astype(np.float32))
    putr("ones", np.ones((128, 1), np.float32))
    putr("lng", np.tile(np.asarray(ln_g, np.float32).reshape(1, 64), (4, 1)))
    putr("lnb", np.tile(np.asarray(ln_b, np.float32).reshape(1, 64), (4, 1)))
    return cpc, cpr


def build_kernel(n_local=4, S=4096, n_cores=N_CORES):
    C = 256
    NK = n_local * 2
    SC = S // 512
    m_total = n_cores * n_local * S
    n_total_imgs = n_cores * n_local

    nc = bacc.Bacc("TRN2", target_bir_lowering=False, num_devices=n_cores)

    Xd = nc.declare_dram_parameter("X", [n_local, 2, 128, S], F32, isOutput=False)
    outd = nc.declare_dram_parameter("out", [n_local, 2, 128, S], BF16, isOutput=True)
    cpcd = nc.declare_dram_parameter("cpack_crit", [128, CW_CRIT], F32,
                                     isOutput=False)
    cprd = nc.declare_dram_parameter("cpack_rest", [128, CW_REST], F32,
                                     isOutput=False)

    with tile.TileContext(nc) as tc:
        _build_tile(tc, Xd, outd, cpcd, cprd, n_local=n_local, S=S,
                    n_cores=n_cores, C=C, NK=NK, SC=SC, m_total=m_total,
                    n_total_imgs=n_total_imgs)
    nc.finalize()
    return nc


def _build_tile(tc, Xd, outd, cpcd, cprd, *, n_local, S, n_cores, C, NK, SC,
                m_total, n_total_imgs):
    nc = tc.nc
    from contextlib import ExitStack
    ctx = ExitStack()
    with ctx:
        consts = ctx.enter_context(tc.tile_pool(name="consts", bufs=1))
        xb_pool = ctx.enter_context(tc.tile_pool(name="xb", bufs=1))
        stats = ctx.enter_context(tc.tile_pool(name="stats", bufs=1))
        stage_pool = ctx.enter_context(tc.tile_pool(name="stage", bufs=3))
        scr_pool = ctx.enter_context(tc.tile_pool(name="scr", bufs=2))
        small = ctx.enter_context(tc.tile_pool(name="small", bufs=1))
        dram = ctx.enter_context(tc.tile_pool(name="dram", bufs=1, space="DRAM"))

        cpc = consts.tile([128, CW_CRIT], F32)
        nc.sync.dma_start(out=cpc[:], in_=cpcd[:, :])
        cpr = consts.tile([128, CW_REST], F32)

        def csc(name, rows=128):
            c0, c1 = _CRIT_COLS[name]
            return cpc[0:rows, c0:c1]

        def cs(name, rows=128):
            c0, c1 = _REST_COLS[name]
            return cpr[0:rows, c0:c1]

        ident = csc("ident")
        onesrow = csc("onesrow", rows=1)
        xw = csc("xw", rows=1)
        neghalfI = cs("neghalf")
        maskeps2 = cs("maskeps2")
        ioverm2 = cs("ioverm2")
        fc1t = cs("fc1t")
        fc2t = cs("fc2t", rows=64)
        gmask = cs("gmask")
        gmaskT15 = cs("gmaskT15", rows=2)
        ones = cs("ones")
        lng4 = cs("lng", rows=n_local)
        lnb4 = cs("lnb", rows=n_local)

        ident_bf = consts.tile([128, 128], BF16)
        nc.vector.tensor_copy(ident_bf[:], ident)
        neghalf_bf = consts.tile([128, 128], BF16)
        nc.vector.tensor_scalar(out=neghalf_bf[:], in0=ident, scalar1=-0.5,
                                scalar2=None, op0=ALU.mult)

        rs = stats.tile([128, NK], F32)
        rsa = stats.tile([128, NK], F32)
        rsb = stats.tile([128, NK], F32)
        ss = stats.tile([128, NK], F32)
        xv = stats.tile([128, NK], F32)

        # ================= PHASE 1: load + cast + Gram =================
        xb_tiles = []
        pg_pool = tc.tile_pool(name="gram", bufs=1, space="PSUM")
        tp_pool = tc.tile_pool(name="tp", bufs=3, space="PSUM")
        chunk_pool = tc.tile_pool(name="chunk", bufs=4)
        SH = S // 2
        with pg_pool as pgp, tp_pool as tpp, chunk_pool as chp:
            pg = [pgp.tile([128, 128 * n_local], F32, tag=f"pg{h}",
                           name=f"pg{h}") for h in range(2)]
            for k in range(NK):
                h, n = divmod(k, n_local)
                if k == 2:
                    nc.scalar.dma_start(out=cpr[:], in_=cprd[:, :])
                xr = xb_pool.tile([128, S], BF16, tag=f"xb{k}")
                xb_tiles.append(xr)
                for half_i, acc in ((0, rsa), (1, rsb)):
                    xin = stage_pool.tile([128, SH], F32, tag="stage",
                                          name=f"xin{k}_{half_i}")
                    hidx = 2 * k + half_i
                    ldeng = nc.gpsimd if (hidx < 2 or hidx % 2 == 1) else nc.sync
                    ldeng.dma_start(
                        out=xin[:], in_=Xd[n, h][:, SH * half_i:SH * (half_i + 1)])
                    nc.scalar.activation(
                        out=xr[:, SH * half_i:SH * (half_i + 1)], in_=xin[:],
                        func=ACTF.Copy, accum_out=acc[:, k:k + 1])
                for c2 in range(SC // 2):
                    tp = tpp.tile([128, 1024], F32)
                    for q in range(8):
                        col0 = 1024 * c2 + 128 * q
                        nc.tensor.matmul(
                            tp[:, 128 * q:128 * q + 128],
                            lhsT=xr[:, col0:col0 + 128],
                            rhs=ident_bf[:], start=True, stop=True)
                    chbf = chp.tile([128, 1024], BF16)
                    if c2 == 2:
                        nc.scalar.copy(chbf[:], tp[:])
                    else:
                        nc.vector.tensor_copy(chbf[:], tp[:])
                    for q in range(8):
                        nc.tensor.matmul(
                            pg[h][:, 128 * n:128 * n + 128],
                            lhsT=chbf[:, 128 * q:128 * q + 128],
                            rhs=chbf[:, 128 * q:128 * q + 128],
                            start=(c2 == 0 and q == 0),
                            stop=(c2 == SC // 2 - 1 and q == 7))
                nc.vector.tensor_add(rs[:, k:k + 1], rsa[:, k:k + 1],
                                     rsb[:, k:k + 1])
                scr = scr_pool.tile([128, 128], F32)
                nc.vector.tensor_mul(scr[:], pg[h][:, 128 * n:128 * n + 128],
                                     ident)
                nc.vector.tensor_reduce(ss[:, k:k + 1], scr[:], axis=AX.X,
                                        op=ALU.add)

            chs = stats.tile([128, 2], F32)
            for h in range(2):
                nc.vector.tensor_reduce(
                    chs[:, h:h + 1], rs[:, n_local * h:n_local * (h + 1)],
                    axis=AX.X, op=ALU.add)
            sloc = [small.tile([128, 128], F32, tag=f"sloc{h}", name=f"sloc{h}")
                    for h in range(2)]
            for h in range(2):
                nc.vector.tensor_copy(sloc[h][:], pg[h][:, 0:128])
                for nn_ in range(1, n_local):
                    nc.vector.tensor_add(
                        sloc[h][:], sloc[h][:],
                        pg[h][:, 128 * nn_:128 * (nn_ + 1)])

        ssum = small.tile([128, 1], F32)
        nc.vector.tensor_reduce(ssum[:], ss[:], axis=AX.X, op=ALU.add)
        rs2 = small.tile([128, NK], F32)
        nc.vector.tensor_mul(rs2[:], rs[:], rs[:])
        rssum = small.tile([128, 1], F32)
        nc.vector.tensor_reduce(rssum[:], rs2[:], axis=AX.X, op=ALU.add)
        xvr = small.tile([128, 1], F32)
        nc.vector.tensor_scalar(out=xvr[:], in0=rssum[:],
                                scalar1=-1.0 / (S * (S - 1.0)), scalar2=None,
                                op0=ALU.mult)
        nc.vector.tensor_scalar(out=rssum[:], in0=ssum[:],
                                scalar1=1.0 / (S - 1.0), scalar2=None,
                                op0=ALU.mult)
        nc.vector.tensor_add(xvr[:], xvr[:], rssum[:])
        with tc.tile_pool(name="ps_xv", bufs=1, space="PSUM") as pxp:
            ps_xv = pxp.tile([1, 1], F32)
            nc.tensor.matmul(ps_xv[:], lhsT=xvr[:], rhs=ones, start=True,
                             stop=True)
            xvsum = small.tile([1, 1], F32)
            nc.vector.tensor_copy(xvsum[:], ps_xv[:])

        # ================= ALL-REDUCE =================
        PAYW = 133
        pay = small.tile([128, PAYW], F32)
        nc.vector.memset(pay[:, 128:PAYW], 0.0)
        for h in range(2):
            nc.vector.tensor_copy(pay[0:64, 64 * h:64 * h + 64],
                                  sloc[h][0:64, 0:64])
            nc.vector.tensor_copy(pay[64:128, 64 * h:64 * h + 64],
                                  sloc[h][64:128, 64:128])
        nc.vector.tensor_copy(pay[:, 128:130], chs[:])
        nc.vector.tensor_copy(pay[0:1, 130:131], xvsum[:])
        for h in range(2):
            nc.vector.tensor_reduce(pay[:, 131 + h:132 + h],
                                    ss[:, n_local * h:n_local * (h + 1)],
                                    axis=AX.X, op=ALU.add)
        sglob = []
        for h in range(2):
            sg_t = small.tile([128, 128], F32, tag=f"sglob{h}", name=f"sglob{h}")
            nc.vector.memset(sg_t[:], 0.0)
            sglob.append(sg_t)
        ccin = dram.tile([128, PAYW], F32)
        ccout = dram.tile([128, PAYW], F32, addr_space="Shared")
        nc.sync.dma_start(out=ccin[:], in_=pay[:])
        nc.gpsimd.collective_compute(
            "AllReduce", ALU.add,
            replica_groups=[list(range(n_cores))],
            ins=[ccin[:].opt()], outs=[ccout[:].opt()])
        gpay = small.tile([128, PAYW], F32)
        nc.sync.dma_start(out=gpay[:], in_=ccout[:])
        for h in range(2):
            nc.vector.tensor_copy(sglob[h][0:64, 0:64],
                                  gpay[0:64, 64 * h:64 * h + 64])
            nc.vector.tensor_copy(sglob[h][64:128, 64:128],
                                  gpay[64:128, 64 * h:64 * h + 64])

        for k in range(NK):
            t1 = scr_pool.tile([128, 1], F32, tag="t1", name=f"xvt{k}")
            nc.vector.tensor_mul(t1[:], rs[:, k:k + 1], rs[:, k:k + 1])
            nc.vector.tensor_scalar(
                out=t1[:], in0=t1[:], scalar1=1.0 / (S * (S - 1.0)),
                scalar2=None, op0=ALU.mult)
            nc.vector.tensor_scalar(
                out=xv[:, k:k + 1], in0=ss[:, k:k + 1], scalar1=1.0 / (S - 1.0),
                scalar2=None, op0=ALU.mult)
            nc.vector.tensor_sub(xv[:, k:k + 1], xv[:, k:k + 1], t1[:])

        # ============ AR BRANCH MLP ============
        spsum = ctx.enter_context(tc.tile_pool(name="spsum", bufs=2, space="PSUM"))
        h_ps = spsum.tile([n_local, 64], F32, tag="sp")
        for h in range(2):
            nc.tensor.matmul(
                h_ps[:], lhsT=xv[:, n_local * h:n_local * (h + 1)],
                rhs=fc1t[:, 64 * h:64 * h + 64], start=(h == 0), stop=(h == 1))
        h_sb = small.tile([n_local, 64], F32)
        nc.vector.tensor_copy(h_sb[:], h_ps[:])
        bst = small.tile([n_local, 6], F32)
        nc.vector.bn_stats(out=bst[:], in_=h_sb[:])
        mv = small.tile([n_local, 2], F32)
        nc.vector.bn_aggr(out=mv[:], in_=bst[:])
        ve = small.tile([n_local, 1], F32)
        nc.vector.tensor_scalar(out=ve[:], in0=mv[:, 1:2], scalar1=LN_EPS,
                                scalar2=None, op0=ALU.add)
        s0 = small.tile([n_local, 1], F32)
        nc.scalar.activation(out=s0[:], in_=ve[:], func=ACTF.Sqrt)
        rstd = small.tile([n_local, 1], F32)
        nc.vector.reciprocal(rstd[:], s0[:])
        hln = small.tile([n_local, 64], F32)
        nc.vector.tensor_scalar(out=hln[:], in0=h_sb[:], scalar1=mv[:, 0:1],
                                scalar2=rstd[:], op0=ALU.subtract, op1=ALU.mult)
        nc.vector.tensor_mul(hln[:], hln[:], lng4)
        nc.vector.tensor_add(hln[:], hln[:], lnb4)
        nc.vector.tensor_scalar_max(hln[:], hln[:], 0.0)
        hT_ps = spsum.tile([64, n_local], F32, tag="sp")
        nc.tensor.transpose(hT_ps[:], hln[:], ident[0:n_local, 0:n_local])
        hT = small.tile([64, n_local], F32)
        nc.vector.tensor_copy(hT[:], hT_ps[:])
        y_ps = spsum.tile([n_local, 256], F32, tag="sp")
        nc.tensor.matmul(y_ps[:], lhsT=hT[:], rhs=fc2t, start=True, stop=True)
        y_sb = small.tile([n_local, 256], F32)
        nc.scalar.activation(out=y_sb[:], in_=y_ps[:], func=ACTF.Sigmoid)
        yT = small.tile([128, NK], F32)
        for h in range(2):
            yT_ps = spsum.tile([128, n_local], F32, tag="sp")
            nc.tensor.transpose(yT_ps[:], y_sb[:, 128 * h:128 * h + 128],
                                ident[0:n_local, 0:n_local])
            nc.vector.tensor_copy(yT[:, n_local * h:n_local * (h + 1)], yT_ps[:])
        w_sb = small.tile([1, 1], F32)
        nc.scalar.activation(out=w_sb[:], in_=xw, func=ACTF.Sigmoid)
        onemw = small.tile([1, 1], F32)
        nc.vector.tensor_scalar(out=onemw[:], in0=w_sb[:], scalar1=-1.0,
                                scalar2=1.0, op0=ALU.mult, op1=ALU.add)

        # ============ POST-ALLREDUCE ============
        xvm = small.tile([1, 1], F32)
        nc.vector.tensor_scalar(out=xvm[:], in0=gpay[0:1, 130:131],
                                scalar1=1.0 / (n_total_imgs * C), scalar2=None,
                                op0=ALU.mult)
        sq0 = small.tile([1, 1], F32)
        nc.scalar.activation(out=sq0[:], in_=xvm[:], func=ACTF.Sqrt)
        rscale = small.tile([1, 1], F32)
        nc.vector.reciprocal(rscale[:], sq0[:])
        yscs = small.tile([1, 1], F32)
        nc.vector.tensor_mul(yscs[:], onemw[:], rscale[:])
        wcol = small.tile([128, 1], F32)
        yscol = small.tile([128, 1], F32)
        with tc.tile_pool(name="bc_ps", bufs=2, space="PSUM") as bcp:
            w_ps = bcp.tile([128, 1], F32)
            nc.tensor.matmul(w_ps[:], lhsT=onesrow, rhs=w_sb[:], start=True,
                             stop=True)
            nc.vector.tensor_copy(wcol[:], w_ps[:])
            y_ps2 = bcp.tile([128, 1], F32)
            nc.tensor.matmul(y_ps2[:], lhsT=onesrow, rhs=yscs[:], start=True,
                             stop=True)
            nc.vector.tensor_copy(yscol[:], y_ps2[:])
        yT2 = small.tile([128, NK], F32)
        nc.vector.tensor_scalar(out=yT2[:], in0=yT[:], scalar1=yscol[:],
                                scalar2=None, op0=ALU.mult)

        # ---- Sigma both halves fused; traces via payload sums ----
        with tc.tile_pool(name="sg_ps", bufs=1, space="PSUM") as sgp:
            rhs4 = small.tile([128, 4], F32)
            nc.vector.tensor_copy(rhs4[:, 0:2], gpay[:, 131:133])
            nc.vector.tensor_mul(rhs4[:, 2:4], gpay[:, 128:130],
                                 gpay[:, 128:130])
            tr_ps = sgp.tile([2, 4], F32, tag="sg", bufs=2)
            nc.tensor.matmul(tr_ps[:], lhsT=gmask, rhs=rhs4[:], start=True,
                             stop=True)
            tr4 = small.tile([2, 4], F32)
            nc.vector.tensor_copy(tr4[:], tr_ps[:])
            trg = small.tile([2, 2], F32)
            nc.vector.tensor_scalar(out=trg[:], in0=tr4[:, 2:4],
                                    scalar1=-1.0 / m_total, scalar2=None,
                                    op0=ALU.mult)
            nc.vector.tensor_add(trg[:], trg[:], tr4[:, 0:2])
            nc.vector.tensor_scalar(out=trg[:], in0=trg[:], scalar1=EPS,
                                    scalar2=64.0 / m_total, op0=ALU.mult,
                                    op1=ALU.add)
            rtr22 = small.tile([2, 2], F32)
            nc.vector.reciprocal(rtr22[:], trg[:])
            rtr_ps = sgp.tile([128, 2], F32, tag="sg", bufs=2)
            nc.tensor.matmul(rtr_ps[:], lhsT=gmaskT15, rhs=rtr22[:],
                             start=True, stop=True)
            rtrcol2 = small.tile([128, 2], F32)
            nc.vector.tensor_copy(rtrcol2[:], rtr_ps[:])

            chrow = []
            chrow_m = []
            for h in range(2):
                chr_ps = sgp.tile([1, 128], F32, tag="sg", bufs=2,
                                  name=f"chrps{h}")
                nc.tensor.transpose(chr_ps[:], gpay[:, 128 + h:129 + h], ident)
                cr_t = small.tile([1, 128], F32, tag=f"chrow{h}",
                                  name=f"chrow{h}")
                crm_t = small.tile([1, 128], F32, tag=f"chrm{h}",
                                   name=f"chrm{h}")
                nc.vector.tensor_copy(cr_t[:], chr_ps[:])
                nc.scalar.mul(out=crm_t[:], in_=chr_ps[:], mul=1.0 / m_total)
                chrow.append(cr_t)
                chrow_m.append(crm_t)
            u_ps = sgp.tile([128, 256], F32, tag="sgw")
            for h in range(2):
                nc.tensor.matmul(u_ps[:, 128 * h:128 * h + 128],
                                 lhsT=chrow_m[h][:], rhs=chrow[h][:],
                                 start=True, stop=True)
            sig2 = small.tile([128, 256], F32)
            for h in range(2):
                nc.vector.tensor_sub(sig2[:, 128 * h:128 * h + 128],
                                     sglob[h][:], u_ps[:, 128 * h:128 * h + 128])
            nc.vector.tensor_mul(sig2[:], sig2[:], maskeps2)
            nc.vector.tensor_add(sig2[:], sig2[:], ioverm2)
            sig15 = []
            for h in range(2):
                sg15_t = small.tile([128, 128], BF16, tag=f"sig15{h}",
                                    name=f"sig15{h}")
                if h == 0:
                    nc.vector.tensor_scalar(out=sg15_t[:], in0=sig2[:, 0:128],
                                            scalar1=rtrcol2[:, 0:1],
                                            scalar2=None, op0=ALU.mult)
                else:
                    nc.scalar.activation(out=sg15_t[:], in_=sig2[:, 128:256],
                                         func=ACTF.Copy,
                                         scale=rtrcol2[:, 1:2])
                sig15.append(sg15_t)
            P = []
            p2t = []
            pxt = []
            for h in range(2):
                p_t = small.tile([128, 128], BF16, tag=f"P{h}", name=f"P{h}")
                nc.vector.tensor_add(p_t[:], sig15[h][:], neghalf_bf[:])
                P.append(p_t)
                p2t.append(small.tile([128, 128], BF16, tag=f"p2{h}",
                                      name=f"p2{h}"))
                pxt.append(small.tile([128, 128], BF16, tag=f"px{h}",
                                      name=f"px{h}"))
            for it in range(1, T_NEWTON):
                ps_a = sgp.tile([128, 256], F32, tag="sgw", name=f"nwa{it}")
                for h in range(2):
                    nc.tensor.matmul(ps_a[:, 128 * h:128 * h + 128],
                                     lhsT=P[h][:], rhs=P[h][:], start=True,
                                     stop=True)
                ps_b = sgp.tile([128, 256], F32, tag="sgw2", name=f"nwb{it}")
                for h in range(2):
                    nc.tensor.matmul(ps_b[:, 128 * h:128 * h + 128],
                                     lhsT=P[h][:], rhs=sig15[h][:], start=True,
                                     stop=True)
                nc.vector.tensor_copy(p2t[0][:], ps_a[:, 0:128])
                nc.scalar.copy(p2t[1][:], ps_a[:, 128:256])
                nc.vector.tensor_copy(pxt[0][:], ps_b[:, 0:128])
                nc.scalar.copy(pxt[1][:], ps_b[:, 128:256])
                ps_c = sgp.tile([128, 256], F32, tag="sgw3", name=f"nwc{it}")
                for h in range(2):
                    nc.tensor.matmul(ps_c[:, 128 * h:128 * h + 128],
                                     lhsT=p2t[h][:], rhs=pxt[h][:],
                                     start=True, stop=False)
                    nc.tensor.matmul(ps_c[:, 128 * h:128 * h + 128],
                                     lhsT=P[h][:], rhs=neghalf_bf[:],
                                     start=False, stop=True)
                nc.vector.tensor_copy(P[0][:], ps_c[:, 0:128])
                nc.scalar.copy(P[1][:], ps_c[:, 128:256])

        mw = []
        for h in range(2):
            t = small.tile([128, 128], F32, tag=f"mw{h}")
            nc.vector.tensor_scalar(out=t[:], in0=P[h][:], scalar1=wcol[:],
                                    scalar2=None, op0=ALU.mult)
            mw.append(t)

        # ============ PHASE 2: fused bf16 apply ============
        mpool = ctx.enter_context(tc.tile_pool(name="mts", bufs=1))
        dtile_pool = ctx.enter_context(tc.tile_pool(name="dtile", bufs=2))
        ostage_pool = ctx.enter_context(tc.tile_pool(name="ostage", bufs=2))
        with tc.tile_pool(name="apply_ps", bufs=3, space="PSUM") as app:
            for k in range(NK):
                h, n = divmod(k, n_local)
                dtile = dtile_pool.tile([128, 128], F32)
                nc.vector.tensor_scalar(out=dtile[:], in0=ident,
                                        scalar1=yT2[:, k:k + 1], scalar2=None,
                                        op0=ALU.mult)
                nc.vector.tensor_add(dtile[:], dtile[:], mw[h][:])
                m_b = mpool.tile([128, 128], BF16, tag=f"m{k}")
                nc.vector.tensor_copy(m_b[:], dtile[:])
                ost = ostage_pool.tile([128, S], BF16, tag="ostage",
                                       name=f"ost{k}")
                for j in range(S // 1024):
                    ap = app.tile([128, 1024], F32)
                    for jj in range(2):
                        c0 = 512 * jj
                        nc.tensor.matmul(
                            ap[:, c0:c0 + 512], lhsT=m_b[:],
                            rhs=xb_tiles[k][:, 1024 * j + c0:1024 * j + c0 + 512],
                            start=True, stop=True)
                    if j % 2 == 0:
                        nc.vector.tensor_copy(ost[:, 1024 * j:1024 * (j + 1)],
                                              ap[:])
                    else:
                        nc.scalar.copy(ost[:, 1024 * j:1024 * (j + 1)], ap[:])
                    if j % 2 == 1:
                        steng = nc.sync if (2 * k + j // 2) % 2 == 0 else nc.gpsimd
                        steng.dma_start(
                            out=outd[n, h][:, SH * (j // 2):SH * (j // 2 + 1)],
                            in_=ost[:, SH * (j // 2):SH * (j // 2 + 1)])


_KERNEL_CACHE = {}


def _get_kernel(n_local=4, S=4096):
    key = (n_local, S)
    if key not in _KERNEL_CACHE:
        _KERNEL_CACHE[key] = build_kernel(n_local=n_local, S=S)
    return _KERNEL_CACHE[key]


def _make_in_maps(inputs, n_local=4, S=4096):
    X = np.asarray(inputs["X"], dtype=np.float32)
    m_total = X.shape[0] * S
    cpc, cpr = _consts_pack(np.asarray(inputs["fc1_w"], np.float32),
                            np.asarray(inputs["fc2_w"], np.float32),
                            np.asarray(inputs["ln_g"], np.float32),
                            np.asarray(inputs["ln_b"], np.float32),
                            np.asarray(inputs["x_weight"], np.float32),
                            m_total)
    in_maps = []
    for i in range(N_CORES):
        shard = X[i * n_local:(i + 1) * n_local].reshape(n_local, 2, 128, S)
        in_maps.append({"X": np.ascontiguousarray(shard),
                        "cpack_crit": cpc, "cpack_rest": cpr})
    return in_maps


def kernel(X, fc1_w, ln_g, ln_b, fc2_w, x_weight):
    X = np.asarray(X, dtype=np.float32)
    N, C, H, W = X.shape
    assert (N, C, H, W) == (32, 256, 64, 64)
    S = H * W
    n_local = N // N_CORES

    nc = _get_kernel()
    in_maps = _make_in_maps(
        {"X": X, "fc1_w": fc1_w, "ln_g": ln_g, "ln_b": ln_b,
         "fc2_w": fc2_w, "x_weight": x_weight}, n_local=n_local, S=S)

    res = bass_utils.run_bass_kernel_spmd(nc, in_maps,
                                          core_ids=list(range(N_CORES)))
    out = np.empty((N, C, H, W), dtype=np.float32)
    for i in range(N_CORES):
        out[i * n_local:(i + 1) * n_local] = np.asarray(
            res.results[i]["out"], dtype=np.float32).reshape(n_local, 256, H, W)
    return out


# revision 48
# speedup vs baseline: 1.0214x; 1.0214x over previous
"""Trainium2 Bass kernel for nn_CE_25872882991735 — v4b reconstruction.

Best-measured variant (166.5us). Phase 1: f32 loads -> ACT bf16 cast with
rowsum accum -> regular-matmul transposes (HAM-warm) -> bf16 Gram per image.
Single 68KB AllReduce; fp32 Newton-Schulz fused both halves; bf16 fused
apply; bf16 output.
"""
import sys

try:
    import concourse.bass as bass  # noqa: F401
except ImportError:  # pragma: no cover
    sys.path.insert(0, "/opt/trn_rl_repo")

import numpy as np

import concourse.bacc as bacc
import concourse.tile as tile
from concourse import mybir
from concourse import bass_utils

F32 = mybir.dt.float32
BF16 = mybir.dt.bfloat16
AX = mybir.AxisListType
ALU = mybir.AluOpType
ACTF = mybir.ActivationFunctionType

N_CORES = 8
EPS = 1e-5
LN_EPS = 1e-5
T_NEWTON = 3

_CRIT_COLS = {}
_REST_COLS = {}


def _build_cols():
    c = 0
    for name, w in [("ident", 128), ("onesrow", 128), ("xw", 1)]:
        _CRIT_COLS[name] = (c, c + w)
        c += w
    cw_crit = c
    c = 0
    for name, w in [("neghalf", 128), ("maskeps2", 256), ("ioverm2", 256),
                    ("fc1t", 128), ("fc2t", 256), ("gmask", 2),
                    ("gmaskT", 128), ("gmaskT15", 128), ("ones", 1),
                    ("lng", 64), ("lnb", 64)]:
        _REST_COLS[name] = (c, c + w)
        c += w
    return cw_crit, c


CW_CRIT, CW_REST = _build_cols()


def _consts_pack(fc1_w, fc2_w, ln_g, ln_b, x_weight, m_total):
    cpc = np.zeros((128, CW_CRIT), np.float32)
    cpr = np.zeros((128, CW_REST), np.float32)

    def putc(name, arr):
        c0, c1 = _CRIT_COLS[name]
        cpc[:arr.shape[0], c0:c1] = arr

    def putr(name, arr):
        c0, c1 = _REST_COLS[name]
        cpr[:arr.shape[0], c0:c1] = arr

    ident = np.eye(128, dtype=np.float32)
    putc("ident", ident)
    putc("onesrow", np.ones((1, 128), np.float32))
    putc("xw", np.asarray(x_weight, np.float32).reshape(1, 1))

    putr("neghalf", (-0.5 * ident).astype(np.float32))
    blk = np.zeros((128, 128), np.float32)
    blk[:64, :64] = EPS
    blk[64:, 64:] = EPS
    putr("maskeps2", np.concatenate([blk, blk], axis=1))
    iov = ident * (1.0 / m_total)
    putr("ioverm2", np.concatenate([iov, iov], axis=1))
    f1 = np.ascontiguousarray(fc1_w.T).reshape(2, 128, 64)
    f1p = np.zeros((128, 128), np.float32)
    f1p[:, 0:64] = f1[0]
    f1p[:, 64:128] = f1[1]
    putr("fc1t", f1p)
    f2 = np.zeros((64, 256), np.float32)
    f2[:, :] = fc2_w.T
    putr("fc2t", f2)
    gmask = np.zeros((128, 2), np.float32)
    gmask[:64, 0] = 1.0
    gmask[64:, 1] = 1.0
    putr("gmask", gmask)
    putr("gmaskT", gmask.T)
    putr("gmaskT15", (1.5 * gmask.T).astype(np.float32))
    putr("ones", np.ones((128, 1), np.float32))
    putr("lng", np.tile(np.asarray(ln_g, np.float32).reshape(1, 64), (4, 1)))
    putr("lnb", np.tile(np.asarray(ln_b, np.float32).reshape(1, 64), (4, 1)))
    return cpc, cpr


def build_kernel(n_local=4, S=4096, n_cores=N_CORES):
    C = 256
    NK = n_local * 2
    SC = S // 512
    m_total = n_cores * n_local * S
    n_total_imgs = n_cores * n_local

    nc = bacc.Bacc("TRN2", target_bir_lowering=False, num_devices=n_cores)

    Xd = nc.declare_dram_parameter("X", [n_local, 2, 128, S], F32, isOutput=False)
    outd = nc.declare_dram_parameter("out", [n_local, 2, 128, S], BF16, isOutput=True)
    cpcd = nc.declare_dram_parameter("cpack_crit", [128, CW_CRIT], F32,
                                     isOutput=False)
    cprd = nc.declare_dram_parameter("cpack_rest", [128, CW_REST], F32,
                                     isOutput=False)

    with tile.TileContext(nc) as tc:
        _build_tile(tc, Xd, outd, cpcd, cprd, n_local=n_local, S=S,
                    n_cores=n_cores, C=C, NK=NK, SC=SC, m_total=m_total,
                    n_total_imgs=n_total_imgs)
    nc.finalize()
    return nc


def _build_tile(tc, Xd, outd, cpcd, cprd, *, n_local, S, n_cores, C, NK, SC,
                m_total, n_total_imgs):
    nc = tc.nc
    from contextlib import ExitStack
    ctx = ExitStack()
    with ctx:
        consts = ctx.enter_context(tc.tile_pool(name="consts", bufs=1))
        xb_pool = ctx.enter_context(tc.tile_pool(name="xb", bufs=1))
        stats = ctx.enter_context(tc.tile_pool(name="stats", bufs=1))
        stage_pool = ctx.enter_context(tc.tile_pool(name="stage", bufs=4))
        scr_pool = ctx.enter_context(tc.tile_pool(name="scr", bufs=2))
        small = ctx.enter_context(tc.tile_pool(name="small", bufs=1))
        dram = ctx.enter_context(tc.tile_pool(name="dram", bufs=1, space="DRAM"))

        cpc = consts.tile([128, CW_CRIT], F32)
        nc.sync.dma_start(out=cpc[:], in_=cpcd[:, :])
        cpr = consts.tile([128, CW_REST], F32)

        def csc(name, rows=128):
            c0, c1 = _CRIT_COLS[name]
            return cpc[0:rows, c0:c1]

        def cs(name, rows=128):
            c0, c1 = _REST_COLS[name]
            return cpr[0:rows, c0:c1]

        ident = csc("ident")
        onesrow = csc("onesrow", rows=1)
        xw = csc("xw", rows=1)
        neghalfI = cs("neghalf")
        maskeps2 = cs("maskeps2")
        ioverm2 = cs("ioverm2")
        fc1t = cs("fc1t")
        fc2t = cs("fc2t", rows=64)
        gmask = cs("gmask")
        gmaskT15 = cs("gmaskT15", rows=2)
        ones = cs("ones")
        lng4 = cs("lng", rows=n_local)
        lnb4 = cs("lnb", rows=n_local)

        ident_bf = consts.tile([128, 128], BF16)
        nc.vector.tensor_copy(ident_bf[:], ident)

        rs = stats.tile([128, NK], F32)
        rsa = stats.tile([128, NK], F32)
        rsb = stats.tile([128, NK], F32)
        ss = stats.tile([128, NK], F32)
        xv = stats.tile([128, NK], F32)

        # ================= PHASE 1: load + cast + Gram =================
        xb_tiles = []
        pg_pool = tc.tile_pool(name="gram", bufs=1, space="PSUM")
        tp_pool = tc.tile_pool(name="tp", bufs=3, space="PSUM")
        chunk_pool = tc.tile_pool(name="chunk", bufs=6)
        SH = S // 2
        with pg_pool as pgp, tp_pool as tpp, chunk_pool as chp:
            pg = [pgp.tile([128, 128 * n_local], F32, tag=f"pg{h}",
                           name=f"pg{h}") for h in range(2)]
            for k in range(NK):
                h, n = divmod(k, n_local)
                if k == 2:
                    nc.scalar.dma_start(out=cpr[:], in_=cprd[:, :])
                xr = xb_pool.tile([128, S], BF16, tag=f"xb{k}")
                xb_tiles.append(xr)
                for half_i, acc in ((0, rsa), (1, rsb)):
                    xin = stage_pool.tile([128, SH], F32, tag="stage",
                                          name=f"xin{k}_{half_i}")
                    hidx = 2 * k + half_i
                    ldeng = nc.gpsimd if (hidx < 2 or hidx % 2 == 1) else nc.sync
                    ldeng.dma_start(
                        out=xin[:], in_=Xd[n, h][:, SH * half_i:SH * (half_i + 1)])
                    nc.scalar.activation(
                        out=xr[:, SH * half_i:SH * (half_i + 1)], in_=xin[:],
                        func=ACTF.Copy, accum_out=acc[:, k:k + 1])
                for c2 in range(SC // 2):
                    tp = tpp.tile([128, 1024], F32)
                    for q in range(8):
                        col0 = 1024 * c2 + 128 * q
                        nc.tensor.matmul(
                            tp[:, 128 * q:128 * q + 128],
                            lhsT=xr[:, col0:col0 + 128],
                            rhs=ident_bf[:], start=True, stop=True)
                    chbf = chp.tile([128, 1024], BF16)
                    if c2 == 2:
                        nc.scalar.copy(chbf[:], tp[:])
                    else:
                        nc.vector.tensor_copy(chbf[:], tp[:])
                    for q in range(8):
                        nc.tensor.matmul(
                            pg[h][:, 128 * n:128 * n + 128],
                            lhsT=chbf[:, 128 * q:128 * q + 128],
                            rhs=chbf[:, 128 * q:128 * q + 128],
                            start=(c2 == 0 and q == 0),
                            stop=(c2 == SC // 2 - 1 and q == 7))
                nc.vector.tensor_add(rs[:, k:k + 1], rsa[:, k:k + 1],
                                     rsb[:, k:k + 1])
                scr = scr_pool.tile([128, 128], F32)
                nc.vector.tensor_mul(scr[:], pg[h][:, 128 * n:128 * n + 128],
                                     ident)
                nc.vector.tensor_reduce(ss[:, k:k + 1], scr[:], axis=AX.X,
                                        op=ALU.add)

            chs = stats.tile([128, 2], F32)
            for h in range(2):
                nc.vector.tensor_reduce(
                    chs[:, h:h + 1], rs[:, n_local * h:n_local * (h + 1)],
                    axis=AX.X, op=ALU.add)
            sloc = [small.tile([128, 128], F32, tag=f"sloc{h}", name=f"sloc{h}")
                    for h in range(2)]
            for h in range(2):
                nc.vector.tensor_copy(sloc[h][:], pg[h][:, 0:128])
                for nn_ in range(1, n_local):
                    nc.vector.tensor_add(
                        sloc[h][:], sloc[h][:],
                        pg[h][:, 128 * nn_:128 * (nn_ + 1)])

        ssum = small.tile([128, 1], F32)
        nc.vector.tensor_reduce(ssum[:], ss[:], axis=AX.X, op=ALU.add)
        rs2 = small.tile([128, NK], F32)
        nc.vector.tensor_mul(rs2[:], rs[:], rs[:])
        rssum = small.tile([128, 1], F32)
        nc.vector.tensor_reduce(rssum[:], rs2[:], axis=AX.X, op=ALU.add)
        xvr = small.tile([128, 1], F32)
        nc.vector.tensor_scalar(out=xvr[:], in0=rssum[:],
                                scalar1=-1.0 / (S * (S - 1.0)), scalar2=None,
                                op0=ALU.mult)
        nc.vector.tensor_scalar(out=rssum[:], in0=ssum[:],
                                scalar1=1.0 / (S - 1.0), scalar2=None,
                                op0=ALU.mult)
        nc.vector.tensor_add(xvr[:], xvr[:], rssum[:])
        with tc.tile_pool(name="ps_xv", bufs=1, space="PSUM") as pxp:
            ps_xv = pxp.tile([1, 1], F32)
            nc.tensor.matmul(ps_xv[:], lhsT=xvr[:], rhs=ones, start=True,
                             stop=True)
            xvsum = small.tile([1, 1], F32)
            nc.vector.tensor_copy(xvsum[:], ps_xv[:])

        # ================= ALL-REDUCE =================
        PAYW = 133
        pay = small.tile([128, PAYW], F32)
        nc.vector.memset(pay[:, 128:PAYW], 0.0)
        for h in range(2):
            nc.vector.tensor_copy(pay[0:64, 64 * h:64 * h + 64],
                                  sloc[h][0:64, 0:64])
            nc.vector.tensor_copy(pay[64:128, 64 * h:64 * h + 64],
                                  sloc[h][64:128, 64:128])
        nc.vector.tensor_copy(pay[:, 128:130], chs[:])
        nc.vector.tensor_copy(pay[0:1, 130:131], xvsum[:])
        for h in range(2):
            nc.vector.tensor_reduce(pay[:, 131 + h:132 + h],
                                    ss[:, n_local * h:n_local * (h + 1)],
                                    axis=AX.X, op=ALU.add)
        sglob = []
        for h in range(2):
            sg_t = small.tile([128, 128], F32, tag=f"sglob{h}", name=f"sglob{h}")
            nc.vector.memset(sg_t[:], 0.0)
            sglob.append(sg_t)
        ccin = dram.tile([128, PAYW], F32)
        ccout = dram.tile([128, PAYW], F32, addr_space="Shared")
        nc.sync.dma_start(out=ccin[:], in_=pay[:])
        nc.gpsimd.collective_compute(
            "AllReduce", ALU.add,
            replica_groups=[list(range(n_cores))],
            ins=[ccin[:].opt()], outs=[ccout[:].opt()])
        gpay = small.tile([128, PAYW], F32)
        nc.sync.dma_start(out=gpay[:], in_=ccout[:])
        for h in range(2):
            nc.vector.tensor_copy(sglob[h][0:64, 0:64],
                                  gpay[0:64, 64 * h:64 * h + 64])
            nc.vector.tensor_copy(sglob[h][64:128, 64:128],
                                  gpay[64:128, 64 * h:64 * h + 64])

        for k in range(NK):
            t1 = scr_pool.tile([128, 1], F32, tag="t1", name=f"xvt{k}")
            nc.vector.tensor_mul(t1[:], rs[:, k:k + 1], rs[:, k:k + 1])
            nc.vector.tensor_scalar(
                out=t1[:], in0=t1[:], scalar1=1.0 / (S * (S - 1.0)),
                scalar2=None, op0=ALU.mult)
            nc.vector.tensor_scalar(
                out=xv[:, k:k + 1], in0=ss[:, k:k + 1], scalar1=1.0 / (S - 1.0),
                scalar2=None, op0=ALU.mult)
            nc.vector.tensor_sub(xv[:, k:k + 1], xv[:, k:k + 1], t1[:])

        # ============ AR BRANCH MLP ============
        spsum = ctx.enter_context(tc.tile_pool(name="spsum", bufs=2, space="PSUM"))
        h_ps = spsum.tile([n_local, 64], F32, tag="sp")
        for h in range(2):
            nc.tensor.matmul(
                h_ps[:], lhsT=xv[:, n_local * h:n_local * (h + 1)],
                rhs=fc1t[:, 64 * h:64 * h + 64], start=(h == 0), stop=(h == 1))
        h_sb = small.tile([n_local, 64], F32)
        nc.vector.tensor_copy(h_sb[:], h_ps[:])
        bst = small.tile([n_local, 6], F32)
        nc.vector.bn_stats(out=bst[:], in_=h_sb[:])
        mv = small.tile([n_local, 2], F32)
        nc.vector.bn_aggr(out=mv[:], in_=bst[:])
        ve = small.tile([n_local, 1], F32)
        nc.vector.tensor_scalar(out=ve[:], in0=mv[:, 1:2], scalar1=LN_EPS,
                                scalar2=None, op0=ALU.add)
        s0 = small.tile([n_local, 1], F32)
        nc.scalar.activation(out=s0[:], in_=ve[:], func=ACTF.Sqrt)
        rstd = small.tile([n_local, 1], F32)
        nc.vector.reciprocal(rstd[:], s0[:])
        hln = small.tile([n_local, 64], F32)
        nc.vector.tensor_scalar(out=hln[:], in0=h_sb[:], scalar1=mv[:, 0:1],
                                scalar2=rstd[:], op0=ALU.subtract, op1=ALU.mult)
        nc.vector.tensor_mul(hln[:], hln[:], lng4)
        nc.vector.tensor_add(hln[:], hln[:], lnb4)
        nc.vector.tensor_scalar_max(hln[:], hln[:], 0.0)
        hT_ps = spsum.tile([64, n_local], F32, tag="sp")
        nc.tensor.transpose(hT_ps[:], hln[:], ident[0:n_local, 0:n_local])
        hT = small.tile([64, n_local], F32)
        nc.vector.tensor_copy(hT[:], hT_ps[:])
        y_ps = spsum.tile([n_local, 256], F32, tag="sp")
        nc.tensor.matmul(y_ps[:], lhsT=hT[:], rhs=fc2t, start=True, stop=True)
        y_sb = small.tile([n_local, 256], F32)
        nc.scalar.activation(out=y_sb[:], in_=y_ps[:], func=ACTF.Sigmoid)
        yT = small.tile([128, NK], F32)
        for h in range(2):
            yT_ps = spsum.tile([128, n_local], F32, tag="sp")
            nc.tensor.transpose(yT_ps[:], y_sb[:, 128 * h:128 * h + 128],
                                ident[0:n_local, 0:n_local])
            nc.vector.tensor_copy(yT[:, n_local * h:n_local * (h + 1)], yT_ps[:])
        w_sb = small.tile([1, 1], F32)
        nc.scalar.activation(out=w_sb[:], in_=xw, func=ACTF.Sigmoid)
        onemw = small.tile([1, 1], F32)
        nc.vector.tensor_scalar(out=onemw[:], in0=w_sb[:], scalar1=-1.0,
                                scalar2=1.0, op0=ALU.mult, op1=ALU.add)

        # ============ POST-ALLREDUCE ============
        xvm = small.tile([1, 1], F32)
        nc.vector.tensor_scalar(out=xvm[:], in0=gpay[0:1, 130:131],
                                scalar1=1.0 / (n_total_imgs * C), scalar2=None,
                                op0=ALU.mult)
        sq0 = small.tile([1, 1], F32)
        nc.scalar.activation(out=sq0[:], in_=xvm[:], func=ACTF.Sqrt)
        rscale = small.tile([1, 1], F32)
        nc.vector.reciprocal(rscale[:], sq0[:])
        yscs = small.tile([1, 1], F32)
        nc.vector.tensor_mul(yscs[:], onemw[:], rscale[:])
        wcol = small.tile([128, 1], F32)
        yscol = small.tile([128, 1], F32)
        with tc.tile_pool(name="bc_ps", bufs=2, space="PSUM") as bcp:
            w_ps = bcp.tile([128, 1], F32)
            nc.tensor.matmul(w_ps[:], lhsT=onesrow, rhs=w_sb[:], start=True,
                             stop=True)
            nc.vector.tensor_copy(wcol[:], w_ps[:])
            y_ps2 = bcp.tile([128, 1], F32)
            nc.tensor.matmul(y_ps2[:], lhsT=onesrow, rhs=yscs[:], start=True,
                             stop=True)
            nc.vector.tensor_copy(yscol[:], y_ps2[:])
        yT2 = small.tile([128, NK], F32)
        nc.vector.tensor_scalar(out=yT2[:], in0=yT[:], scalar1=yscol[:],
                                scalar2=None, op0=ALU.mult)

        # ---- Sigma both halves fused; traces via payload sums ----
        with tc.tile_pool(name="sg_ps", bufs=1, space="PSUM") as sgp:
            rhs4 = small.tile([128, 4], F32)
            nc.vector.tensor_copy(rhs4[:, 0:2], gpay[:, 131:133])
            nc.vector.tensor_mul(rhs4[:, 2:4], gpay[:, 128:130],
                                 gpay[:, 128:130])
            tr_ps = sgp.tile([2, 4], F32, tag="sg", bufs=2)
            nc.tensor.matmul(tr_ps[:], lhsT=gmask, rhs=rhs4[:], start=True,
                             stop=True)
            tr4 = small.tile([2, 4], F32)
            nc.vector.tensor_copy(tr4[:], tr_ps[:])
            trg = small.tile([2, 2], F32)
            nc.vector.tensor_scalar(out=trg[:], in0=tr4[:, 2:4],
                                    scalar1=-1.0 / m_total, scalar2=None,
                                    op0=ALU.mult)
            nc.vector.tensor_add(trg[:], trg[:], tr4[:, 0:2])
            nc.vector.tensor_scalar(out=trg[:], in0=trg[:], scalar1=EPS,
                                    scalar2=64.0 / m_total, op0=ALU.mult,
                                    op1=ALU.add)
            rtr22 = small.tile([2, 2], F32)
            nc.vector.reciprocal(rtr22[:], trg[:])
            rtr_ps = sgp.tile([128, 2], F32, tag="sg", bufs=2)
            nc.tensor.matmul(rtr_ps[:], lhsT=gmaskT15, rhs=rtr22[:],
                             start=True, stop=True)
            rtrcol2 = small.tile([128, 2], F32)
            nc.vector.tensor_copy(rtrcol2[:], rtr_ps[:])

            chrow = []
            chrow_m = []
            for h in range(2):
                chr_ps = sgp.tile([1, 128], F32, tag="sg", bufs=2,
                                  name=f"chrps{h}")
                nc.tensor.transpose(chr_ps[:], gpay[:, 128 + h:129 + h], ident)
                cr_t = small.tile([1, 128], F32, tag=f"chrow{h}",
                                  name=f"chrow{h}")
                crm_t = small.tile([1, 128], F32, tag=f"chrm{h}",
                                   name=f"chrm{h}")
                nc.vector.tensor_copy(cr_t[:], chr_ps[:])
                nc.scalar.mul(out=crm_t[:], in_=chr_ps[:], mul=1.0 / m_total)
                chrow.append(cr_t)
                chrow_m.append(crm_t)
            u_ps = sgp.tile([128, 256], F32, tag="sgw")
            for h in range(2):
                nc.tensor.matmul(u_ps[:, 128 * h:128 * h + 128],
                                 lhsT=chrow_m[h][:], rhs=chrow[h][:],
                                 start=True, stop=True)
            sig2 = small.tile([128, 256], F32)
            for h in range(2):
                nc.vector.tensor_sub(sig2[:, 128 * h:128 * h + 128],
                                     sglob[h][:], u_ps[:, 128 * h:128 * h + 128])
            nc.vector.tensor_mul(sig2[:], sig2[:], maskeps2)
            nc.vector.tensor_add(sig2[:], sig2[:], ioverm2)
            sig15 = []
            for h in range(2):
                sg15_t = small.tile([128, 128], F32, tag=f"sig15{h}",
                                    name=f"sig15{h}")
                if h == 0:
                    nc.vector.tensor_scalar(out=sg15_t[:], in0=sig2[:, 0:128],
                                            scalar1=rtrcol2[:, 0:1],
                                            scalar2=None, op0=ALU.mult)
                else:
                    nc.scalar.activation(out=sg15_t[:], in_=sig2[:, 128:256],
                                         func=ACTF.Copy,
                                         scale=rtrcol2[:, 1:2])
                sig15.append(sg15_t)
            P = []
            p2t = []
            pxt = []
            for h in range(2):
                p_t = small.tile([128, 128], F32, tag=f"P{h}", name=f"P{h}")
                nc.vector.tensor_add(p_t[:], sig15[h][:], neghalfI)
                P.append(p_t)
                p2t.append(small.tile([128, 128], F32, tag=f"p2{h}",
                                      name=f"p2{h}"))
                pxt.append(small.tile([128, 128], F32, tag=f"px{h}",
                                      name=f"px{h}"))
            for it in range(1, T_NEWTON):
                ps_a = sgp.tile([128, 256], F32, tag="sgw", name=f"nwa{it}")
                for h in range(2):
                    nc.tensor.matmul(ps_a[:, 128 * h:128 * h + 128],
                                     lhsT=P[h][:], rhs=P[h][:], start=True,
                                     stop=True)
                ps_b = sgp.tile([128, 256], F32, tag="sgw2", name=f"nwb{it}")
                for h in range(2):
                    nc.tensor.matmul(ps_b[:, 128 * h:128 * h + 128],
                                     lhsT=P[h][:], rhs=sig15[h][:], start=True,
                                     stop=True)
                nc.vector.tensor_copy(p2t[0][:], ps_a[:, 0:128])
                nc.scalar.copy(p2t[1][:], ps_a[:, 128:256])
                nc.vector.tensor_copy(pxt[0][:], ps_b[:, 0:128])
                nc.scalar.copy(pxt[1][:], ps_b[:, 128:256])
                ps_c = sgp.tile([128, 256], F32, tag="sgw3", name=f"nwc{it}")
                for h in range(2):
                    nc.tensor.matmul(ps_c[:, 128 * h:128 * h + 128],
                                     lhsT=p2t[h][:], rhs=pxt[h][:],
                                     start=True, stop=False)
                    nc.tensor.matmul(ps_c[:, 128 * h:128 * h + 128],
                                     lhsT=P[h][:], rhs=neghalfI,
                                     start=False, stop=True)
                nc.vector.tensor_copy(P[0][:], ps_c[:, 0:128])
                nc.scalar.copy(P[1][:], ps_c[:, 128:256])

        mw = []
        for h in range(2):
            t = small.tile([128, 128], F32, tag=f"mw{h}")
            nc.vector.tensor_scalar(out=t[:], in0=P[h][:], scalar1=wcol[:],
                                    scalar2=None, op0=ALU.mult)
            mw.append(t)

        # ============ PHASE 2: fused bf16 apply ============
        mpool = ctx.enter_context(tc.tile_pool(name="mts", bufs=1))
        dtile_pool = ctx.enter_context(tc.tile_pool(name="dtile", bufs=2))
        ostage_pool = ctx.enter_context(tc.tile_pool(name="ostage", bufs=3))
        with tc.tile_pool(name="apply_ps", bufs=3, space="PSUM") as app:
            for k in range(NK):
                h, n = divmod(k, n_local)
                dtile = dtile_pool.tile([128, 128], F32)
                nc.vector.tensor_scalar(out=dtile[:], in0=ident,
                                        scalar1=yT2[:, k:k + 1], scalar2=None,
                                        op0=ALU.mult)
                nc.vector.tensor_add(dtile[:], dtile[:], mw[h][:])
                m_b = mpool.tile([128, 128], BF16, tag=f"m{k}")
                nc.vector.tensor_copy(m_b[:], dtile[:])
                ost = ostage_pool.tile([128, S], BF16, tag="ostage",
                                       name=f"ost{k}")
                for j in range(S // 1024):
                    ap = app.tile([128, 1024], F32)
                    for jj in range(2):
                        c0 = 512 * jj
                        nc.tensor.matmul(
                            ap[:, c0:c0 + 512], lhsT=m_b[:],
                            rhs=xb_tiles[k][:, 1024 * j + c0:1024 * j + c0 + 512],
                            start=True, stop=True)
                    if j % 2 == 0:
                        nc.vector.tensor_copy(ost[:, 1024 * j:1024 * (j + 1)],
                                              ap[:])
                    else:
                        nc.scalar.copy(ost[:, 1024 * j:1024 * (j + 1)], ap[:])
                    if j % 2 == 1:
                        steng = nc.sync if (2 * k + j // 2) % 2 == 0 else nc.gpsimd
                        steng.dma_start(
                            out=outd[n, h][:, SH * (j // 2):SH * (j // 2 + 1)],
                            in_=ost[:, SH * (j // 2):SH * (j // 2 + 1)])


_KERNEL_CACHE = {}


def _get_kernel(n_local=4, S=4096):
    key = (n_local, S)
    if key not in _KERNEL_CACHE:
        _KERNEL_CACHE[key] = build_kernel(n_local=n_local, S=S)
    return _KERNEL_CACHE[key]


def _make_in_maps(inputs, n_local=4, S=4096):
    X = np.asarray(inputs["X"], dtype=np.float32)
    m_total = X.shape[0] * S
    cpc, cpr = _consts_pack(np.asarray(inputs["fc1_w"], np.float32),
                            np.asarray(inputs["fc2_w"], np.float32),
                            np.asarray(inputs["ln_g"], np.float32),
                            np.asarray(inputs["ln_b"], np.float32),
                            np.asarray(inputs["x_weight"], np.float32),
                            m_total)
    in_maps = []
    for i in range(N_CORES):
        shard = X[i * n_local:(i + 1) * n_local].reshape(n_local, 2, 128, S)
        in_maps.append({"X": np.ascontiguousarray(shard),
                        "cpack_crit": cpc, "cpack_rest": cpr})
    return in_maps


def kernel(X, fc1_w, ln_g, ln_b, fc2_w, x_weight):
    X = np.asarray(X, dtype=np.float32)
    N, C, H, W = X.shape
    assert (N, C, H, W) == (32, 256, 64, 64)
    S = H * W
    n_local = N // N_CORES

    nc = _get_kernel()
    in_maps = _make_in_maps(
        {"X": X, "fc1_w": fc1_w, "ln_g": ln_g, "ln_b": ln_b,
         "fc2_w": fc2_w, "x_weight": x_weight}, n_local=n_local, S=S)

    res = bass_utils.run_bass_kernel_spmd(nc, in_maps,
                                          core_ids=list(range(N_CORES)))
    out = np.empty((N, C, H, W), dtype=np.float32)
    for i in range(N_CORES):
        out[i * n_local:(i + 1) * n_local] = np.asarray(
            res.results[i]["out"], dtype=np.float32).reshape(n_local, 256, H, W)
    return out
